# revision 28
# baseline (speedup 1.0000x reference)
"""Trainium2 Bass kernel for nn_Decoder (GRU decoder + MLP + vocab softmax).

Sharding (8 NeuronCores):
  - GRU + 2-layer MLP: data-parallel over batch (4 examples/core).
    Local tokens are b-major (col = b*128 + t) so the global token index
    G = 512*rank + b*128 + t equals example*128 + t, matching output rows.
  - Final [512,32000] vocab projection + softmax: column-parallel
    (4000 vocab cols/core) with AllReduce'd softmax denominators.

Pipelined structure: the MLP + h2 AllGather run in 4 time-quarters so
they overlap the GRU scan, and the vocab projection for gathered
quarters is interleaved into the scan's gate-latency gaps (one
4-matmul x 500-col chunk per step).  Softmax denominators all-reduce
in 9 small rounds; exp tiles are scaled in place and written out bf16.

Compute dtypes: bf16 matmul operands, fp32 PSUM accumulation and gates,
bf16 hidden state / exp store / output (~3e-3 rel err vs the fp32
reference; gate is 2e-2).  The softmax skips max-subtraction: logits
are O(+-2), far inside exp's fp32 range, and exp+rowsum are fused in
one ScalarE pass via accum_out.
"""

import numpy as np

import concourse.bass as bass
import concourse.tile as tile
from concourse import bacc, mybir
from concourse.bass import ds, ts
from concourse.bass_utils import run_bass_kernel_spmd
from concourse.masks import make_identity

P = 128
NCORES = 8
B, T, E, H, V = 32, 128, 256, 512, 32000
BL = B // NCORES            # 4 examples per core
NTOK = BL * T               # 512 local tokens
G = B * T                   # 4096 global tokens
VS = V // NCORES            # 4000 vocab cols per core
KO = H // P                 # 4 hidden chunks
MO3 = 3 * H // P            # 12 gate chunks (z:0-3, r:4-7, h:8-11)
SO = (E + H) // P           # 6 input chunks
NJ = 8                      # vocab sub-chunks per token tile (8 x 500)
VC = VS // NJ               # 500
NQG = 4                     # gather quarters (time-sliced)
TQS = T // NQG              # 32 timesteps per quarter
BLT = BL * TQS              # 128 tokens per (quarter, rank) tile
NTILE = NQG * NCORES        # 32 token tiles
ROUND_SIZES = [4, 3, 4, 3, 4, 3, 4, 3, 3, 1]  # tiles per softmax AR round
ROUNDS = len(ROUND_SIZES)
TPR = max(ROUND_SIZES)

f32 = mybir.dt.float32
bf16 = mybir.dt.bfloat16
fp8 = mybir.dt.float8e4

R_FP8 = False
SKIP_SCALE = False

TRACE = False
TRACE_KWARGS = {}
LAST_RESULT = None

RG = [list(range(NCORES))]

# scan-step at which quarter q's vocab chunks may start (AG latency margin)
VOC_START = [32 * q + 57 for q in range(NQG)]


def _build(has_b3: bool, has_gb: bool, debug: str | None = None):
    nc = bacc.Bacc("TRN2", target_bir_lowering=False, debug=False,
                   num_devices=NCORES)

    enc_ext = nc.dram_tensor("encoder_input", [BL, T, E], f32, kind="ExternalInput").ap()
    dec_ext = nc.dram_tensor("decoder_input", [BL, H], f32, kind="ExternalInput").ap()
    gk_ext = nc.dram_tensor("gru_kernel", [E + H, 3 * H], f32, kind="ExternalInput").ap()
    gr_ext = nc.dram_tensor("gru_rec_kernel", [H, 3 * H], f32, kind="ExternalInput").ap()
    gb_ext = nc.dram_tensor("gru_bias", [2, 3 * H], f32, kind="ExternalInput").ap()
    w1_ext = nc.dram_tensor("w1", [H, H], f32, kind="ExternalInput").ap()
    b1_ext = nc.dram_tensor("b1", [H], f32, kind="ExternalInput").ap()
    w2_ext = nc.dram_tensor("w2", [H, H], f32, kind="ExternalInput").ap()
    b2_ext = nc.dram_tensor("b2", [H], f32, kind="ExternalInput").ap()
    w3_ext = nc.dram_tensor("w3", [H, VS], f32, kind="ExternalInput").ap()
    b3_ext = nc.dram_tensor("b3", [VS], f32, kind="ExternalInput").ap()

    out_ext = nc.dram_tensor("out", [NTILE, P, VS], bf16, kind="ExternalOutput").ap()
    dbg_ext = None
    if debug == "xproj":
        dbg_ext = nc.dram_tensor("dbg", [P, MO3, NTOK], f32, kind="ExternalOutput").ap()
    elif debug == "hseq":
        dbg_ext = nc.dram_tensor("dbg", [P, KO, NTOK], bf16, kind="ExternalOutput").ap()
    elif debug == "h2loc":
        dbg_ext = nc.dram_tensor("dbg", [P, KO, NTOK], bf16, kind="ExternalOutput").ap()
    elif debug == "h2g":
        dbg_ext = nc.dram_tensor("dbg", [NQG, P, KO * NCORES * BLT], bf16,
                                 kind="ExternalOutput").ap()
    elif debug == "w3b":
        dbg_ext = nc.dram_tensor("dbg", [P, KO * VS], bf16,
                                 kind="ExternalOutput").ap()

    with tile.TileContext(nc) as tc:
        with tc.tile_pool(name="dram", bufs=1, space="DRAM") as dram_pool:
            h2_bounce = [dram_pool.tile([P, KO * BL * TQS], bf16, name=f"h2b_{q}")
                         for q in range(NQG)]
            h2_gath = [dram_pool.tile([NCORES * P, KO * BL * TQS], bf16,
                                      addr_space="Shared",
                                      name=f"h2g_{q}") for q in range(NQG)]
            sums_in = [dram_pool.tile([P * ROUND_SIZES[r]], f32,
                                      name=f"sums_in_{r}")
                       for r in range(ROUNDS)]
            sums_out = [dram_pool.tile([P * ROUND_SIZES[r]], f32,
                                       addr_space="Shared",
                                       name=f"sums_out_{r}")
                        for r in range(ROUNDS)]
            _build_body(nc, tc, has_b3, has_gb, debug, dbg_ext,
                        enc_ext, dec_ext, gk_ext, gr_ext, gb_ext,
                        w1_ext, b1_ext, w2_ext, b2_ext, w3_ext, b3_ext,
                        out_ext, h2_bounce, h2_gath, sums_in, sums_out)
    nc.finalize()
    return nc


def _build_body(nc, tc, has_b3, has_gb, debug, dbg_ext,
                enc_ext, dec_ext, gk_ext, gr_ext, gb_ext,
                w1_ext, b1_ext, w2_ext, b2_ext, w3_ext, b3_ext,
                out_ext, h2_bounce, h2_gath, sums_in, sums_out):
    from contextlib import ExitStack

    Ident = mybir.ActivationFunctionType.Identity
    Sig = mybir.ActivationFunctionType.Sigmoid
    Relu = mybir.ActivationFunctionType.Relu
    Exp = mybir.ActivationFunctionType.Exp

    persist = ExitStack()
    wpool = persist.enter_context(tc.tile_pool(name="wpool", bufs=1))
    w3b = wpool.tile([P, KO, VS], bf16)
    b3bc = wpool.tile([P, VS], f32, name="b3bc") if has_b3 else None

    gru_stack = ExitStack()
    gpool = gru_stack.enter_context(tc.tile_pool(name="gpool", bufs=1))
    w1b = gpool.tile([P, KO, H], bf16)
    w2b = gpool.tile([P, KO, H], bf16)
    b1T = gpool.tile([P, KO], f32)
    b2T = gpool.tile([P, KO], f32)
    h2T = gpool.tile([P, KO, NTOK], bf16)
    gt_pool = gru_stack.enter_context(tc.tile_pool(name="gt", bufs=3))
    psum_pro = gru_stack.enter_context(tc.tile_pool(name="ps_pro", bufs=2, space="PSUM"))
    psum_rec = gru_stack.enter_context(tc.tile_pool(name="ps_rec", bufs=1, space="PSUM"))

    Rdt = fp8 if R_FP8 else bf16
    Rb = gpool.tile([P, KO, 3 * H], Rdt)
    Wkb = gpool.tile([P, SO, 3 * H], bf16)
    seqT = gpool.tile([P, SO, NTOK], bf16)
    xprojT = gpool.tile([P, MO3, NTOK], bf16)
    hseqT = gpool.tile([P, KO, NTOK], bf16)
    h1T = gpool.tile([P, KO, NTOK], bf16)
    ident = gpool.tile([P, P], bf16)
    wtmp_pool = gru_stack.enter_context(tc.tile_pool(name="wtmp", bufs=2))

    def load_cast(dst3, src_ext, nck, width, tag):
        src_c = src_ext.rearrange("(k p) m -> k p m", p=P)
        for k in range(nck):
            tmp = wtmp_pool.tile([P, width], f32, tag=tag, name=f"{tag}_{k}")
            nc.sync.dma_start(out=tmp[:], in_=src_c[k])
            nc.vector.tensor_copy(out=dst3[:, k], in_=tmp[:])

    # ---- encoder/decoder prep (short-lived tiles; space reused by vocab) ----
    make_identity(nc, ident)
    seqT4 = seqT.rearrange("p so (b t) -> p so b t", b=BL)
    enc_stack = ExitStack()
    epool = enc_stack.enter_context(tc.tile_pool(name="epool", bufs=1))
    enc_nat = epool.tile([P, BL, E], f32)
    nc.sync.dma_start(out=enc_nat[:], in_=enc_ext.rearrange("b t c -> t b c"))
    enc_natb = epool.tile([P, BL, E], bf16)
    nc.vector.tensor_copy(out=enc_natb[:], in_=enc_nat[:])
    for b in range(BL):
        for co in range(E // P):
            pst = psum_pro.tile([P, P], bf16, tag="pro", name=f"tp_{b}_{co}")
            nc.tensor.transpose(pst[:], enc_natb[:, b, ts(co, P)], ident)
            nc.vector.tensor_copy(out=seqT4[:, co, b, :], in_=pst[:])
    # decoder -> bf16, broadcast over t into seqT chunks 2-5
    decT = epool.tile([P, KO, BL], f32)
    for b in range(BL):
        nc.sync.dma_start(out=decT[:, :, b],
                          in_=dec_ext[b].rearrange("(ko p) -> p ko", p=P))
    decTb = epool.tile([P, KO, BL], bf16)
    nc.vector.tensor_copy(out=decTb[:], in_=decT[:])
    nc.vector.tensor_copy(out=seqT4[:, E // P:SO],
                          in_=decTb[:, :, :, None].to_broadcast((P, KO, BL, T)))
    enc_stack.close()

    # ---- vocab-phase pools (allocated after enc_stack frees its space) ----
    voc_stack = ExitStack()
    vpool = voc_stack.enter_context(tc.tile_pool(name="vpool", bufs=1))
    h2g = [vpool.tile([P, KO, NCORES, BLT], bf16, name=f"h2gs_{q}")
           for q in range(NQG)]
    exp_pool = voc_stack.enter_context(tc.tile_pool(name="exp", bufs=7))
    sc_pool = voc_stack.enter_context(tc.tile_pool(name="scp", bufs=3))
    psum_voc = voc_stack.enter_context(tc.tile_pool(name="ps_voc", bufs=3, space="PSUM"))

    load_cast(Wkb, gk_ext, SO, 3 * H, "wtmp")
    load_cast(Rb, gr_ext, KO, 3 * H, "wtmp")

    # gru biases (generic path; skipped when zero)
    if has_gb:
        gbT = gpool.tile([P, MO3, 2], f32)
        for i in range(2):
            nc.sync.dma_start(out=gbT[:, :, i],
                              in_=gb_ext[i].rearrange("(mo p) -> p mo", p=P))
        xbias = gpool.tile([P, MO3], f32)
        nc.vector.tensor_copy(out=xbias[:], in_=gbT[:, :, 0])
        nc.vector.tensor_add(out=xbias[:, 0:8], in0=xbias[:, 0:8], in1=gbT[:, 0:8, 1])
        brecH = gpool.tile([P, KO, BL], f32)
        nc.vector.tensor_copy(out=brecH[:],
                              in_=gbT[:, 8:12, 1:2].to_broadcast((P, KO, BL)))

    # ---- deferred weight-load tasks (interleaved into the scan) ----
    w1_c = w1_ext.rearrange("(k p) m -> k p m", p=P)
    w2_c = w2_ext.rearrange("(k p) m -> k p m", p=P)
    w3_c = w3_ext.rearrange("(k p) m -> k p m", p=P)
    W3P = 4            # w3 column pieces per k-chunk
    W3W = VS // W3P    # 1000

    def mk_wtask(dst3, src_c, k, lo, w, tag, nm):
        def run():
            tmp = wtmp_pool.tile([P, w], f32, tag=tag, name=nm)
            nc.sync.dma_start(out=tmp[:], in_=src_c[k][:, ds(lo, w)])
            nc.vector.tensor_copy(out=dst3[:, k, ds(lo, w)], in_=tmp[:])
        return run

    def mk_btask():
        def run():
            nc.sync.dma_start(out=b1T[:], in_=b1_ext.rearrange("(mo p) -> p mo", p=P))
            nc.sync.dma_start(out=b2T[:], in_=b2_ext.rearrange("(mo p) -> p mo", p=P))
            if has_b3:
                b3_brd = bass.AP(tensor=b3_ext.tensor, offset=b3_ext.offset,
                                 ap=[[0, P]] + list(b3_ext.ap))
                nc.sync.dma_start(out=b3bc[:], in_=b3_brd)
        return run

    wload_tasks = []
    for k in range(KO):
        wload_tasks.append(mk_wtask(w1b, w1_c, k, 0, H, "wtmp", f"w1t_{k}"))
    for k in range(KO):
        wload_tasks.append(mk_wtask(w2b, w2_c, k, 0, H, "wtmp", f"w2t_{k}"))
    wload_tasks.append(mk_btask())
    for k in range(KO):
        for pc in range(W3P):
            wload_tasks.append(mk_wtask(w3b, w3_c, k, W3W * pc, W3W, "wtmp",
                                        f"w3t_{k}_{pc}"))
    wload_tasks.reverse()  # pop() from the end

    # ---- x_proj^T = Wk^T @ seq^T (+bias), one m-group per scan step ----
    XC = NQG
    XS = T // XC  # 32 steps per chunk
    seq_bt = seqT.rearrange("p so (b t) -> p so b t", b=BL)
    xp_bt = xprojT.rearrange("p m (b t) -> p m b t", b=BL)

    def emit_xproj_group(c, m):
        ps = psum_pro.tile([P, BL * XS], f32, tag="pro", name=f"xp_{c}_{m}")
        for k in range(SO):
            nc.tensor.matmul(ps[:], lhsT=Wkb[:, k, ts(m, P)],
                             rhs=seq_bt[:, k, :, ds(XS * c, XS)],
                             start=(k == 0), stop=(k == SO - 1))
        dst = xp_bt[:, m, :, ds(XS * c, XS)]
        if has_gb:
            nc.scalar.activation(out=dst, in_=ps[:], func=Ident,
                                 bias=xbias[:, m:m + 1])
        else:
            nc.scalar.copy(out=dst, in_=ps[:])

    for m in range(MO3):
        emit_xproj_group(0, m)

    if debug == "xproj":
        for c in range(1, XC):
            for m in range(MO3):
                emit_xproj_group(c, m)
        nc.sync.dma_start(out=dbg_ext, in_=xprojT[:])

    # ---- MLP (per time-quarter, emitted one m-group per scan step) ----
    hs4 = hseqT.rearrange("p ko (b t) -> p ko b t", b=BL)
    h1_4 = h1T.rearrange("p ko (b t) -> p ko b t", b=BL)
    h2_4 = h2T.rearrange("p ko (b t) -> p ko b t", b=BL)

    def emit_mlp_group(q, g):
        if g < KO:
            m, wsrc, xsrc, dst, bsrc = g, w1b, hs4, h1_4, b1T
        else:
            m, wsrc, xsrc, dst, bsrc = g - KO, w2b, h1_4, h2_4, b2T
        ps = psum_pro.tile([P, BLT], f32, tag="pro", name=f"mlp_{q}_{g}")
        for k in range(KO):
            nc.tensor.matmul(ps[:], lhsT=wsrc[:, k, ts(m, P)],
                             rhs=xsrc[:, k, :, ds(TQS * q, TQS)],
                             start=(k == 0), stop=(k == KO - 1))
        nc.scalar.activation(out=dst[:, m, :, ds(TQS * q, TQS)],
                             in_=ps.rearrange("p (b t) -> p b t", b=BL),
                             func=Relu, bias=bsrc[:, m:m + 1])

    def emit_gather(q):
        # bounce cols = (ko, b, t) so the SBUF-side AP merges to <=3 dims
        bq = h2_bounce[q]
        nc.gpsimd.dma_start(
            out=bq.rearrange("p (ko b t) -> p ko b t", ko=KO, b=BL),
            in_=h2_4[:, :, :, ds(TQS * q, TQS)])
        nc.gpsimd.collective_compute(
            "AllGather", mybir.AluOpType.bypass,
            ins=[bq.opt()], outs=[h2_gath[q].opt()],
            replica_groups=RG,
        )
        src = h2_gath[q].rearrange("(r p) (ko bt) -> ko p r bt", p=P, ko=KO)
        for ko in range(KO):
            nc.scalar.dma_start(out=h2g[q][:, ko], in_=src[ko])

    # ---- vocab + softmax machinery (chunks interleaved into the scan) ----
    round_of_tile = []
    for rd, sz in enumerate(ROUND_SIZES):
        round_of_tile += [rd] * sz
    assert len(round_of_tile) == NTILE

    vstate = dict(chunk=0, half=0, pv=None, round_tiles=[], sums=None, rnd=0,
                  tick=0, ar_gate=[], scale_tasks=[])

    def emit_scale_tile(task):
        expb, (q, r), rcp_p, i = task
        if not SKIP_SCALE:
            nc.vector.tensor_scalar_mul(expb[:], expb[:], rcp_p[:, i:i + 1])
        nc.sync.dma_start(out=out_ext[q * NCORES + r],
                          in_=expb.rearrange("p j v -> p (j v)"))

    def pop_scale_task():
        # round rd's scales unlock once round rd+1's AllReduce was emitted
        if vstate['scale_tasks'] and len(vstate['ar_gate']) > 1:
            task, rd = vstate['scale_tasks'][0]
            if rd + 1 < vstate['rnd']:
                vstate['scale_tasks'].pop(0)
                emit_scale_tile(task)
                return True
        return False

    def emit_round_finish():
        rnd = vstate['rnd']
        nr = ROUND_SIZES[rnd]
        sums = vstate['sums']
        ssum = sc_pool.tile([P, TPR], f32, tag="ssum", name=f"ssum_{rnd}")
        nc.vector.tensor_reduce(out=ssum[:, :nr], in_=sums[:, :nr, :],
                                axis=mybir.AxisListType.X, op=mybir.AluOpType.add)
        nc.gpsimd.dma_start(out=sums_in[rnd].rearrange("(i p) -> p i", p=P),
                            in_=ssum[:, :nr])
        nc.gpsimd.collective_compute(
            "AllReduce", mybir.AluOpType.add,
            ins=[sums_in[rnd].opt()], outs=[sums_out[rnd].opt()],
            replica_groups=RG,
        )
        rcp = sc_pool.tile([P, TPR], f32, tag="rcp", name=f"rcp_{rnd}")
        nc.scalar.dma_start(out=rcp[:, :nr],
                            in_=sums_out[rnd].rearrange("(i p) -> p i", p=P))
        nc.vector.reciprocal(out=rcp[:, :nr], in_=rcp[:, :nr])
        for i, (expb, qr) in enumerate(vstate['round_tiles']):
            vstate['scale_tasks'].append(((expb, qr, rcp, i), rnd))
        vstate['ar_gate'].append(rnd)
        vstate['round_tiles'] = []
        vstate['sums'] = None
        vstate['rnd'] = rnd + 1

    def emit_vocab_half():
        """Emit half of the next (tile, j) chunk: 2 matmuls; exp on 2nd half."""
        c = vstate['chunk']
        ti, j = divmod(c, NJ)
        q, r = divmod(ti, NCORES)
        half = vstate['half']
        if half == 0:
            if j == 0:
                if vstate['sums'] is None:
                    vstate['sums'] = sc_pool.tile([P, TPR, NJ], f32, tag="sums",
                                                  name=f"sums_{round_of_tile[ti]}")
                expb = exp_pool.tile([P, NJ, VC], bf16, tag="expb",
                                     name=f"expb_{q}_{r}")
                vstate['round_tiles'].append((expb, (q, r)))
            vstate['pv'] = psum_voc.tile([P, VC], f32, tag="pv",
                                         name=f"pv_{q}_{r}_{j}")
        pv = vstate['pv']
        for ko in (0, 1) if half == 0 else (2, 3):
            nc.tensor.matmul(pv[:], lhsT=h2g[q][:, ko, r],
                             rhs=w3b[:, ko, ds(VC * j, VC)],
                             start=(ko == 0),
                             stop=(ko == KO - 1) and not has_b3)
        if half == 0:
            vstate['half'] = 1
            return
        vstate['half'] = 0
        expb = vstate['round_tiles'][-1][0]
        i = len(vstate['round_tiles']) - 1
        if has_b3:
            nc.vector.tensor_add(out=pv[:], in0=pv[:],
                                 in1=b3bc[:, ds(VC * j, VC)])
        nc.scalar.activation(out=expb[:, j], in_=pv[:], func=Exp,
                             accum_out=vstate['sums'][:, i, j:j + 1])
        vstate['chunk'] = c + 1
        if j == NJ - 1 and sum(ROUND_SIZES[:vstate['rnd'] + 1]) == ti + 1:
            emit_round_finish()

    # ---- GRU scan (t-major local tokens) ----
    xp4 = xprojT.rearrange("p m (b t) -> p m b t", b=BL)

    # sigmoid as 1/(1+exp(-x)): keeps ScalarE on the Exp table permanently
    # (vocab exps interleave with gate activations; Sigmoid<->Exp switches
    # would reload the ACT function table at ~1.5us per switch).
    def sigmoid_exp(dst, src):
        nc.scalar.activation(out=dst, in_=src, func=Exp, scale=-1.0)
        nc.vector.tensor_scalar_add(dst, dst, 1.0)
        nc.vector.reciprocal(out=dst, in_=dst)

    # t = 0 (h == 0): z,r = sig(xz), hh = relu(xh [+ r*brecH]), h = (1-z)*hh
    zr0 = gt_pool.tile([P, 8, BL], f32, tag="zr")
    sigmoid_exp(zr0[:], xp4[:, 0:8, :, 0])
    hh0 = gt_pool.tile([P, KO, BL], f32, tag="hh")
    if has_gb:
        nc.vector.tensor_mul(out=hh0[:], in0=zr0[:, 4:8], in1=brecH[:])
        nc.vector.tensor_add(out=hh0[:], in0=hh0[:], in1=xp4[:, 8:12, :, 0])
        nc.vector.tensor_scalar_max(hh0[:], hh0[:], 0.0)
    else:
        nc.vector.tensor_scalar_max(hh0[:], xp4[:, 8:12, :, 0], 0.0)
    d0 = gt_pool.tile([P, KO, BL], f32, tag="d")
    nc.vector.tensor_mul(out=d0[:], in0=zr0[:, 0:4], in1=hh0[:])
    nc.vector.tensor_sub(out=hs4[:, :, :, 0], in0=hh0[:], in1=d0[:])

    for t in range(1, T):
        r_ps = psum_rec.tile([P, KO * BL], f32, tag="r_ps", name=f"rp_{t}")
        h_ps = psum_rec.tile([P, KO * BL], f32, tag="h_ps", name=f"hp_{t}")
        z_ps = psum_rec.tile([P, KO * BL], f32, tag="z_ps", name=f"zp_{t}")
        nc.tensor.matmul(r_ps[:], lhsT=ident, rhs=xp4[:, 4:8, :, t],
                         start=True, stop=False)
        for m in range(4):
            for ko in range(KO):
                nc.tensor.matmul(r_ps[:, ds(BL * m, BL)],
                                 lhsT=Rb[:, ko, ts(4 + m, P)],
                                 rhs=hs4[:, ko, :, t - 1],
                                 start=False, stop=(ko == KO - 1) and (m == 3))
        for m in range(4):
            for ko in range(KO):
                nc.tensor.matmul(h_ps[:, ds(BL * m, BL)],
                                 lhsT=Rb[:, ko, ts(8 + m, P)],
                                 rhs=hs4[:, ko, :, t - 1],
                                 start=(ko == 0), stop=(ko == KO - 1))
        nc.tensor.matmul(z_ps[:], lhsT=ident, rhs=xp4[:, 0:4, :, t],
                         start=True, stop=False)
        for m in range(4):
            for ko in range(KO):
                nc.tensor.matmul(z_ps[:, ds(BL * m, BL)],
                                 lhsT=Rb[:, ko, ts(m, P)],
                                 rhs=hs4[:, ko, :, t - 1],
                                 start=False, stop=(ko == KO - 1) and (m == 3))
        rr = gt_pool.tile([P, KO, BL], f32, tag="rr", name=f"rr_{t}")
        sigmoid_exp(rr[:], r_ps.rearrange("p (m b) -> p m b", b=BL))
        hh = gt_pool.tile([P, KO, BL], f32, tag="hh", name=f"hh_{t}")
        hp4 = h_ps.rearrange("p (m b) -> p m b", b=BL)
        if has_gb:
            nc.vector.tensor_add(out=hh[:], in0=hp4, in1=brecH[:])
            nc.vector.tensor_mul(out=hh[:], in0=rr[:], in1=hh[:])
        else:
            nc.vector.tensor_mul(out=hh[:], in0=rr[:], in1=hp4)
        nc.vector.tensor_add(out=hh[:], in0=hh[:], in1=xp4[:, 8:12, :, t])
        nc.vector.tensor_scalar_max(hh[:], hh[:], 0.0)
        dd = gt_pool.tile([P, KO, BL], f32, tag="d", name=f"d_{t}")
        nc.vector.tensor_sub(out=dd[:], in0=hs4[:, :, :, t - 1], in1=hh[:])
        zz = gt_pool.tile([P, KO, BL], f32, tag="zz", name=f"zz_{t}")
        sigmoid_exp(zz[:], z_ps.rearrange("p (m b) -> p m b", b=BL))
        nc.vector.tensor_mul(out=dd[:], in0=zz[:], in1=dd[:])
        nc.vector.tensor_add(out=hs4[:, :, :, t], in0=hh[:], in1=dd[:])

        # ---- interleaved auxiliary work for this step ----
        if wload_tasks:
            wload_tasks.pop()()
        pe_busy = False
        # x_proj chunk c: 12 m-groups at steps 32c-14 .. 32c-3
        for c in range(1, XC):
            m = t - (32 * c - 14)
            if 0 <= m < MO3 and debug != "xproj":
                emit_xproj_group(c, m)
                pe_busy = True
        # MLP for quarter q: 8 groups at steps 32q+33 .. 32q+40, AG at +41
        for q in range(NQG - 1):
            g = t - (32 * q + 33)
            if 0 <= g < 2 * KO:
                emit_mlp_group(q, g)
                pe_busy = True
            elif g == 2 * KO:
                emit_gather(q)
        # vocab half-chunk backfill + paced softmax scale/output emission
        if not pe_busy:
            avail = 0
            for q in range(NQG):
                if t >= VOC_START[q]:
                    avail = (q + 1) * NCORES * NJ
            if vstate['chunk'] < avail:
                emit_vocab_half()
        pop_scale_task()

    if debug == "hseq":
        nc.sync.dma_start(out=dbg_ext, in_=hseqT[:])

    while wload_tasks:
        wload_tasks.pop()()

    # ---- post-scan: last quarter MLP + gather, then drain vocab chunks ----
    for g in range(2 * KO):
        emit_mlp_group(NQG - 1, g)
    emit_gather(NQG - 1)

    if debug == "h2loc":
        nc.sync.dma_start(out=dbg_ext, in_=h2T[:])
    if debug == "h2g":
        for q in range(NQG):
            nc.sync.dma_start(out=dbg_ext[q],
                              in_=h2g[q].rearrange("p ko r bt -> p (ko r bt)"))
    if debug == "w3b":
        nc.sync.dma_start(out=dbg_ext, in_=w3b.rearrange("p k v -> p (k v)"))

    while vstate['chunk'] < NTILE * NJ:
        emit_vocab_half()
        emit_vocab_half()
        pop_scale_task()
    while vstate['scale_tasks']:
        task, rd = vstate['scale_tasks'].pop(0)
        emit_scale_tile(task)

    voc_stack.close()
    gru_stack.close()
    persist.close()


_BUILD_CACHE = {}


def _get_nc(has_b3: bool, has_gb: bool, debug=None):
    key = (has_b3, has_gb, debug)
    if key not in _BUILD_CACHE:
        _BUILD_CACHE[key] = _build(has_b3, has_gb, debug)
    return _BUILD_CACHE[key]


def _make_in_maps(inputs):
    arrs = {k: np.ascontiguousarray(np.asarray(v, dtype=np.float32))
            for k, v in inputs.items()}
    in_maps = []
    for c in range(NCORES):
        in_maps.append({
            "encoder_input": arrs["encoder_input"][BL * c:BL * (c + 1)],
            "decoder_input": arrs["decoder_input"][BL * c:BL * (c + 1)],
            "gru_kernel": arrs["gru_kernel"],
            "gru_rec_kernel": arrs["gru_rec_kernel"],
            "gru_bias": arrs["gru_bias"],
            "w1": arrs["w1"], "b1": arrs["b1"],
            "w2": arrs["w2"], "b2": arrs["b2"],
            "w3": np.ascontiguousarray(arrs["w3"][:, VS * c:VS * (c + 1)]),
            "b3": np.ascontiguousarray(arrs["b3"][VS * c:VS * (c + 1)]),
        })
    flags = (bool(np.any(arrs["b3"])), bool(np.any(arrs["gru_bias"])))
    return in_maps, flags


def kernel(**inputs):
    global LAST_RESULT
    in_maps, (has_b3, has_gb) = _make_in_maps(inputs)
    nc = _get_nc(has_b3, has_gb)
    res = run_bass_kernel_spmd(nc, in_maps, core_ids=list(range(NCORES)),
                               trace=TRACE, **TRACE_KWARGS)
    LAST_RESULT = res
    full = np.empty((B, T, V), np.float32)
    for c in range(NCORES):
        o = np.asarray(res.results[c]["out"], dtype=np.float32)
        # o[q*8+r, bl*32+tq, v] -> full[4r+bl, 32q+tq, VS*c+v]
        o = o.reshape(NQG, NCORES, BL, TQS, VS)
        full[:, :, VS * c:VS * (c + 1)] = o.transpose(1, 2, 0, 3, 4).reshape(B, T, VS)
    return full


# revision 33
# speedup vs baseline: 1.0483x; 1.0483x over previous
"""Trainium2 Bass kernel for nn_Decoder (GRU decoder + MLP + vocab softmax).

Sharding (8 NeuronCores):
  - GRU + 2-layer MLP: data-parallel over batch (4 examples/core).
    Local tokens are b-major (col = b*128 + t) so the global token index
    G = 512*rank + b*128 + t equals example*128 + t, matching output rows.
  - Final [512,32000] vocab projection + softmax: column-parallel
    (4000 vocab cols/core) with AllReduce'd softmax denominators.

Pipelined structure: the MLP + h2 AllGather run in 4 time-quarters so
they overlap the GRU scan, and the vocab projection for gathered
quarters is interleaved into the scan's gate-latency gaps (one
4-matmul x 500-col chunk per step).  Softmax denominators all-reduce
in 9 small rounds; exp tiles are scaled in place and written out bf16.

Compute dtypes: bf16 matmul operands, fp32 PSUM accumulation and gates,
bf16 hidden state / exp store / output (~3e-3 rel err vs the fp32
reference; gate is 2e-2).  The softmax skips max-subtraction: logits
are O(+-2), far inside exp's fp32 range, and exp+rowsum are fused in
one ScalarE pass via accum_out.
"""

import numpy as np

import concourse.bass as bass
import concourse.tile as tile
from concourse import bacc, mybir
from concourse.bass import ds, ts
from concourse.bass_utils import run_bass_kernel_spmd
from concourse.masks import make_identity

P = 128
NCORES = 8
B, T, E, H, V = 32, 128, 256, 512, 32000
BL = B // NCORES            # 4 examples per core
NTOK = BL * T               # 512 local tokens
G = B * T                   # 4096 global tokens
VS = V // NCORES            # 4000 vocab cols per core
KO = H // P                 # 4 hidden chunks
MO3 = 3 * H // P            # 12 gate chunks (z:0-3, r:4-7, h:8-11)
SO = (E + H) // P           # 6 input chunks
NJ = 8                      # vocab sub-chunks per token tile (8 x 500)
VC = VS // NJ               # 500
NQG = 4                     # gather quarters (time-sliced)
TQS = T // NQG              # 32 timesteps per quarter
BLT = BL * TQS              # 128 tokens per (quarter, rank) tile
NTILE = NQG * NCORES        # 32 token tiles
ROUND_SIZES = [4, 3, 4, 3, 4, 3, 4, 3, 4]  # tiles per softmax AR round
ROUNDS = len(ROUND_SIZES)
TPR = max(ROUND_SIZES)

f32 = mybir.dt.float32
bf16 = mybir.dt.bfloat16
fp8 = mybir.dt.float8e4

R_FP8 = False
SKIP_SCALE = False

TRACE = False
TRACE_KWARGS = {}
LAST_RESULT = None

RG = [list(range(NCORES))]

# scan-step at which quarter q's vocab chunks may start (AG latency margin)
VOC_START = [32 * q + 57 for q in range(NQG)]


def _build(has_b3: bool, has_gb: bool, debug: str | None = None):
    nc = bacc.Bacc("TRN2", target_bir_lowering=False, debug=False,
                   num_devices=NCORES)

    enc_ext = nc.dram_tensor("encoder_input", [BL, T, E], f32, kind="ExternalInput").ap()
    dec_ext = nc.dram_tensor("decoder_input", [BL, H], f32, kind="ExternalInput").ap()
    gk_ext = nc.dram_tensor("gru_kernel", [E + H, 3 * H], f32, kind="ExternalInput").ap()
    gr_ext = nc.dram_tensor("gru_rec_kernel", [H, 3 * H], f32, kind="ExternalInput").ap()
    gb_ext = nc.dram_tensor("gru_bias", [2, 3 * H], f32, kind="ExternalInput").ap()
    w1_ext = nc.dram_tensor("w1", [H, H], f32, kind="ExternalInput").ap()
    b1_ext = nc.dram_tensor("b1", [H], f32, kind="ExternalInput").ap()
    w2_ext = nc.dram_tensor("w2", [H, H], f32, kind="ExternalInput").ap()
    b2_ext = nc.dram_tensor("b2", [H], f32, kind="ExternalInput").ap()
    w3_ext = nc.dram_tensor("w3", [H, VS], f32, kind="ExternalInput").ap()
    b3_ext = nc.dram_tensor("b3", [VS], f32, kind="ExternalInput").ap()

    out_ext = nc.dram_tensor("out", [NTILE, P, VS], bf16, kind="ExternalOutput").ap()
    dbg_ext = None
    if debug == "xproj":
        dbg_ext = nc.dram_tensor("dbg", [P, MO3, NTOK], f32, kind="ExternalOutput").ap()
    elif debug == "hseq":
        dbg_ext = nc.dram_tensor("dbg", [P, KO, NTOK], bf16, kind="ExternalOutput").ap()
    elif debug == "h2loc":
        dbg_ext = nc.dram_tensor("dbg", [P, KO, NTOK], bf16, kind="ExternalOutput").ap()
    elif debug == "h2g":
        dbg_ext = nc.dram_tensor("dbg", [NQG, P, KO * NCORES * BLT], bf16,
                                 kind="ExternalOutput").ap()
    elif debug == "w3b":
        dbg_ext = nc.dram_tensor("dbg", [P, KO * VS], bf16,
                                 kind="ExternalOutput").ap()

    with tile.TileContext(nc) as tc:
        with tc.tile_pool(name="dram", bufs=1, space="DRAM") as dram_pool:
            h2_bounce = [dram_pool.tile([P, KO * BL * TQS], bf16, name=f"h2b_{q}")
                         for q in range(NQG)]
            h2_gath = [dram_pool.tile([NCORES * P, KO * BL * TQS], bf16,
                                      addr_space="Shared",
                                      name=f"h2g_{q}") for q in range(NQG)]
            sums_in = [dram_pool.tile([P * ROUND_SIZES[r]], f32,
                                      name=f"sums_in_{r}")
                       for r in range(ROUNDS)]
            sums_out = [dram_pool.tile([P * ROUND_SIZES[r]], f32,
                                       addr_space="Shared",
                                       name=f"sums_out_{r}")
                        for r in range(ROUNDS)]
            _build_body(nc, tc, has_b3, has_gb, debug, dbg_ext,
                        enc_ext, dec_ext, gk_ext, gr_ext, gb_ext,
                        w1_ext, b1_ext, w2_ext, b2_ext, w3_ext, b3_ext,
                        out_ext, h2_bounce, h2_gath, sums_in, sums_out)
    nc.finalize()
    return nc


def _build_body(nc, tc, has_b3, has_gb, debug, dbg_ext,
                enc_ext, dec_ext, gk_ext, gr_ext, gb_ext,
                w1_ext, b1_ext, w2_ext, b2_ext, w3_ext, b3_ext,
                out_ext, h2_bounce, h2_gath, sums_in, sums_out):
    from contextlib import ExitStack

    Ident = mybir.ActivationFunctionType.Identity
    Sig = mybir.ActivationFunctionType.Sigmoid
    Relu = mybir.ActivationFunctionType.Relu
    Exp = mybir.ActivationFunctionType.Exp

    persist = ExitStack()
    wpool = persist.enter_context(tc.tile_pool(name="wpool", bufs=1))
    w3b = wpool.tile([P, KO, VS], bf16)
    b3bc = wpool.tile([P, VS], f32, name="b3bc") if has_b3 else None

    gru_stack = ExitStack()
    gpool = gru_stack.enter_context(tc.tile_pool(name="gpool", bufs=1))
    w1b = gpool.tile([P, KO, H], bf16)
    w2b = gpool.tile([P, KO, H], bf16)
    b1T = gpool.tile([P, KO], f32)
    b2T = gpool.tile([P, KO], f32)
    h2T = gpool.tile([P, KO, NTOK], bf16)
    gt_pool = gru_stack.enter_context(tc.tile_pool(name="gt", bufs=3))
    psum_pro = gru_stack.enter_context(tc.tile_pool(name="ps_pro", bufs=2, space="PSUM"))
    psum_rec = gru_stack.enter_context(tc.tile_pool(name="ps_rec", bufs=1, space="PSUM"))

    Rdt = fp8 if R_FP8 else bf16
    Rb = gpool.tile([P, KO, 3 * H], Rdt)
    Wkb = gpool.tile([P, SO, 3 * H], bf16)
    seqT = gpool.tile([P, SO, NTOK], bf16)
    xprojT = gpool.tile([P, MO3, NTOK], bf16)
    hseqT = gpool.tile([P, KO, NTOK], bf16)
    h1T = gpool.tile([P, KO, NTOK], bf16)
    ident = gpool.tile([P, P], bf16)
    wtmp_pool = gru_stack.enter_context(tc.tile_pool(name="wtmp", bufs=2))

    def load_cast(dst3, src_ext, nck, width, tag):
        src_c = src_ext.rearrange("(k p) m -> k p m", p=P)
        for k in range(nck):
            tmp = wtmp_pool.tile([P, width], f32, tag=tag, name=f"{tag}_{k}")
            nc.sync.dma_start(out=tmp[:], in_=src_c[k])
            nc.vector.tensor_copy(out=dst3[:, k], in_=tmp[:])

    # ---- encoder/decoder prep (short-lived tiles; space reused by vocab) ----
    make_identity(nc, ident)
    seqT4 = seqT.rearrange("p so (b t) -> p so b t", b=BL)
    enc_stack = ExitStack()
    epool = enc_stack.enter_context(tc.tile_pool(name="epool", bufs=1))
    enc_nat = epool.tile([P, BL, E], f32)
    nc.sync.dma_start(out=enc_nat[:], in_=enc_ext.rearrange("b t c -> t b c"))
    enc_natb = epool.tile([P, BL, E], bf16)
    nc.vector.tensor_copy(out=enc_natb[:], in_=enc_nat[:])
    for b in range(BL):
        for co in range(E // P):
            pst = psum_pro.tile([P, P], bf16, tag="pro", name=f"tp_{b}_{co}")
            nc.tensor.transpose(pst[:], enc_natb[:, b, ts(co, P)], ident)
            nc.vector.tensor_copy(out=seqT4[:, co, b, :], in_=pst[:])
    # decoder -> bf16, broadcast over t into seqT chunks 2-5
    decT = epool.tile([P, KO, BL], f32)
    for b in range(BL):
        nc.sync.dma_start(out=decT[:, :, b],
                          in_=dec_ext[b].rearrange("(ko p) -> p ko", p=P))
    decTb = epool.tile([P, KO, BL], bf16)
    nc.vector.tensor_copy(out=decTb[:], in_=decT[:])
    nc.vector.tensor_copy(out=seqT4[:, E // P:SO],
                          in_=decTb[:, :, :, None].to_broadcast((P, KO, BL, T)))
    enc_stack.close()

    # ---- vocab-phase pools (allocated after enc_stack frees its space) ----
    voc_stack = ExitStack()
    vpool = voc_stack.enter_context(tc.tile_pool(name="vpool", bufs=1))
    h2g = [vpool.tile([P, KO, NCORES, BLT], bf16, name=f"h2gs_{q}")
           for q in range(NQG)]
    exp_pool = voc_stack.enter_context(tc.tile_pool(name="exp", bufs=7))
    sc_pool = voc_stack.enter_context(tc.tile_pool(name="scp", bufs=3))
    psum_voc = voc_stack.enter_context(tc.tile_pool(name="ps_voc", bufs=3, space="PSUM"))

    load_cast(Wkb, gk_ext, SO, 3 * H, "wtmp")
    load_cast(Rb, gr_ext, KO, 3 * H, "wtmp")

    # gru biases (generic path; skipped when zero)
    if has_gb:
        gbT = gpool.tile([P, MO3, 2], f32)
        for i in range(2):
            nc.sync.dma_start(out=gbT[:, :, i],
                              in_=gb_ext[i].rearrange("(mo p) -> p mo", p=P))
        xbias = gpool.tile([P, MO3], f32)
        nc.vector.tensor_copy(out=xbias[:], in_=gbT[:, :, 0])
        nc.vector.tensor_add(out=xbias[:, 0:8], in0=xbias[:, 0:8], in1=gbT[:, 0:8, 1])
        brecH = gpool.tile([P, KO, BL], f32)
        nc.vector.tensor_copy(out=brecH[:],
                              in_=gbT[:, 8:12, 1:2].to_broadcast((P, KO, BL)))

    # ---- deferred weight-load tasks (interleaved into the scan) ----
    w1_c = w1_ext.rearrange("(k p) m -> k p m", p=P)
    w2_c = w2_ext.rearrange("(k p) m -> k p m", p=P)
    w3_c = w3_ext.rearrange("(k p) m -> k p m", p=P)
    W3P = 4            # w3 column pieces per k-chunk
    W3W = VS // W3P    # 1000

    def mk_wtask(dst3, src_c, k, lo, w, tag, nm):
        def run():
            tmp = wtmp_pool.tile([P, w], f32, tag=tag, name=nm)
            nc.sync.dma_start(out=tmp[:], in_=src_c[k][:, ds(lo, w)])
            nc.vector.tensor_copy(out=dst3[:, k, ds(lo, w)], in_=tmp[:])
        return run

    def mk_btask():
        def run():
            nc.sync.dma_start(out=b1T[:], in_=b1_ext.rearrange("(mo p) -> p mo", p=P))
            nc.sync.dma_start(out=b2T[:], in_=b2_ext.rearrange("(mo p) -> p mo", p=P))
            if has_b3:
                b3_brd = bass.AP(tensor=b3_ext.tensor, offset=b3_ext.offset,
                                 ap=[[0, P]] + list(b3_ext.ap))
                nc.sync.dma_start(out=b3bc[:], in_=b3_brd)
        return run

    wload_tasks = []
    for k in range(KO):
        wload_tasks.append(mk_wtask(w1b, w1_c, k, 0, H, "wtmp", f"w1t_{k}"))
    for k in range(KO):
        wload_tasks.append(mk_wtask(w2b, w2_c, k, 0, H, "wtmp", f"w2t_{k}"))
    wload_tasks.append(mk_btask())
    for k in range(KO):
        for pc in range(W3P):
            wload_tasks.append(mk_wtask(w3b, w3_c, k, W3W * pc, W3W, "wtmp",
                                        f"w3t_{k}_{pc}"))
    wload_tasks.reverse()  # pop() from the end

    # ---- x_proj^T = Wk^T @ seq^T (+bias), one m-group per scan step ----
    XC = NQG
    XS = T // XC  # 32 steps per chunk
    seq_bt = seqT.rearrange("p so (b t) -> p so b t", b=BL)
    xp_bt = xprojT.rearrange("p m (b t) -> p m b t", b=BL)

    def emit_xproj_group(c, m):
        ps = psum_pro.tile([P, BL * XS], f32, tag="pro", name=f"xp_{c}_{m}")
        for k in range(SO):
            nc.tensor.matmul(ps[:], lhsT=Wkb[:, k, ts(m, P)],
                             rhs=seq_bt[:, k, :, ds(XS * c, XS)],
                             start=(k == 0), stop=(k == SO - 1))
        dst = xp_bt[:, m, :, ds(XS * c, XS)]
        if has_gb:
            nc.scalar.activation(out=dst, in_=ps[:], func=Ident,
                                 bias=xbias[:, m:m + 1])
        else:
            nc.scalar.copy(out=dst, in_=ps[:])

    for m in range(MO3):
        emit_xproj_group(0, m)

    if debug == "xproj":
        for c in range(1, XC):
            for m in range(MO3):
                emit_xproj_group(c, m)
        nc.sync.dma_start(out=dbg_ext, in_=xprojT[:])

    # ---- MLP (per time-quarter, emitted one m-group per scan step) ----
    hs4 = hseqT.rearrange("p ko (b t) -> p ko b t", b=BL)
    h1_4 = h1T.rearrange("p ko (b t) -> p ko b t", b=BL)
    h2_4 = h2T.rearrange("p ko (b t) -> p ko b t", b=BL)

    def emit_mlp_group(q, g):
        if g < KO:
            m, wsrc, xsrc, dst, bsrc = g, w1b, hs4, h1_4, b1T
        else:
            m, wsrc, xsrc, dst, bsrc = g - KO, w2b, h1_4, h2_4, b2T
        ps = psum_pro.tile([P, BLT], f32, tag="pro", name=f"mlp_{q}_{g}")
        for k in range(KO):
            nc.tensor.matmul(ps[:], lhsT=wsrc[:, k, ts(m, P)],
                             rhs=xsrc[:, k, :, ds(TQS * q, TQS)],
                             start=(k == 0), stop=(k == KO - 1))
        nc.scalar.activation(out=dst[:, m, :, ds(TQS * q, TQS)],
                             in_=ps.rearrange("p (b t) -> p b t", b=BL),
                             func=Relu, bias=bsrc[:, m:m + 1])

    def emit_gather(q):
        # bounce cols = (ko, b, t) so the SBUF-side AP merges to <=3 dims
        bq = h2_bounce[q]
        nc.gpsimd.dma_start(
            out=bq.rearrange("p (ko b t) -> p ko b t", ko=KO, b=BL),
            in_=h2_4[:, :, :, ds(TQS * q, TQS)])
        nc.gpsimd.collective_compute(
            "AllGather", mybir.AluOpType.bypass,
            ins=[bq.opt()], outs=[h2_gath[q].opt()],
            replica_groups=RG,
        )
        src = h2_gath[q].rearrange("(r p) (ko bt) -> ko p r bt", p=P, ko=KO)
        for ko in range(KO):
            nc.scalar.dma_start(out=h2g[q][:, ko], in_=src[ko])

    # ---- vocab + softmax machinery (chunks interleaved into the scan) ----
    round_of_tile = []
    for rd, sz in enumerate(ROUND_SIZES):
        round_of_tile += [rd] * sz
    assert len(round_of_tile) == NTILE

    vstate = dict(chunk=0, half=0, pv=None, round_tiles=[], sums=None, rnd=0,
                  tick=0, ar_gate=[], scale_tasks=[])

    def emit_scale_tile(task):
        expb, (q, r), rcp_p, i = task
        if not SKIP_SCALE:
            nc.vector.tensor_scalar_mul(expb[:], expb[:], rcp_p[:, i:i + 1])
        nc.sync.dma_start(out=out_ext[q * NCORES + r],
                          in_=expb.rearrange("p j v -> p (j v)"))

    def pop_scale_task():
        # round rd's scales unlock once rd+1's AllReduce was emitted, or ~28
        # emission ticks after rd's own AllReduce (AR latency has elapsed, so
        # the DVE won't stall on rcp and exp slots free a round earlier)
        if vstate['scale_tasks']:
            task, rd, artick = vstate['scale_tasks'][0]
            if rd + 1 < vstate['rnd'] or vstate['tick'] - artick >= 28:
                vstate['scale_tasks'].pop(0)
                emit_scale_tile(task)
                return True
        return False

    def emit_round_finish():
        rnd = vstate['rnd']
        nr = ROUND_SIZES[rnd]
        sums = vstate['sums']
        ssum = sc_pool.tile([P, TPR], f32, tag="ssum", name=f"ssum_{rnd}")
        nc.vector.tensor_reduce(out=ssum[:, :nr], in_=sums[:, :nr, :],
                                axis=mybir.AxisListType.X, op=mybir.AluOpType.add)
        nc.gpsimd.dma_start(out=sums_in[rnd].rearrange("(i p) -> p i", p=P),
                            in_=ssum[:, :nr])
        nc.gpsimd.collective_compute(
            "AllReduce", mybir.AluOpType.add,
            ins=[sums_in[rnd].opt()], outs=[sums_out[rnd].opt()],
            replica_groups=RG,
        )
        rcp = sc_pool.tile([P, TPR], f32, tag="rcp", name=f"rcp_{rnd}")
        nc.scalar.dma_start(out=rcp[:, :nr],
                            in_=sums_out[rnd].rearrange("(i p) -> p i", p=P))
        nc.vector.reciprocal(out=rcp[:, :nr], in_=rcp[:, :nr])
        for i, (expb, qr) in enumerate(vstate['round_tiles']):
            vstate['scale_tasks'].append(((expb, qr, rcp, i), rnd,
                                          vstate['tick']))
        vstate['ar_gate'].append(rnd)
        vstate['round_tiles'] = []
        vstate['sums'] = None
        vstate['rnd'] = rnd + 1

    def emit_vocab_half():
        """Emit half of the next (tile, j) chunk: 2 matmuls; exp on 2nd half."""
        c = vstate['chunk']
        ti, j = divmod(c, NJ)
        q, r = divmod(ti, NCORES)
        half = vstate['half']
        if half == 0:
            if j == 0:
                if vstate['sums'] is None:
                    vstate['sums'] = sc_pool.tile([P, TPR, NJ], f32, tag="sums",
                                                  name=f"sums_{round_of_tile[ti]}")
                expb = exp_pool.tile([P, NJ, VC], bf16, tag="expb",
                                     name=f"expb_{q}_{r}")
                vstate['round_tiles'].append((expb, (q, r)))
            vstate['pv'] = psum_voc.tile([P, VC], f32, tag="pv",
                                         name=f"pv_{q}_{r}_{j}")
        pv = vstate['pv']
        for ko in (0, 1) if half == 0 else (2, 3):
            nc.tensor.matmul(pv[:], lhsT=h2g[q][:, ko, r],
                             rhs=w3b[:, ko, ds(VC * j, VC)],
                             start=(ko == 0),
                             stop=(ko == KO - 1) and not has_b3)
        if half == 0:
            vstate['half'] = 1
            return
        vstate['half'] = 0
        expb = vstate['round_tiles'][-1][0]
        i = len(vstate['round_tiles']) - 1
        if has_b3:
            nc.vector.tensor_add(out=pv[:], in0=pv[:],
                                 in1=b3bc[:, ds(VC * j, VC)])
        nc.scalar.activation(out=expb[:, j], in_=pv[:], func=Exp,
                             accum_out=vstate['sums'][:, i, j:j + 1])
        vstate['chunk'] = c + 1
        if j == NJ - 1 and sum(ROUND_SIZES[:vstate['rnd'] + 1]) == ti + 1:
            emit_round_finish()

    # ---- GRU scan (t-major local tokens) ----
    xp4 = xprojT.rearrange("p m (b t) -> p m b t", b=BL)

    # sigmoid as 1/(1+exp(-x)): keeps ScalarE on the Exp table permanently
    # (vocab exps interleave with gate activations; Sigmoid<->Exp switches
    # would reload the ACT function table at ~1.5us per switch).
    def sigmoid_exp(dst, src):
        nc.scalar.activation(out=dst, in_=src, func=Exp, scale=-1.0)
        nc.vector.tensor_scalar_add(dst, dst, 1.0)
        nc.vector.reciprocal(out=dst, in_=dst)

    # t = 0 (h == 0): z,r = sig(xz), hh = relu(xh [+ r*brecH]), h = (1-z)*hh
    zr0 = gt_pool.tile([P, 8, BL], f32, tag="zr")
    sigmoid_exp(zr0[:], xp4[:, 0:8, :, 0])
    hh0 = gt_pool.tile([P, KO, BL], f32, tag="hh")
    if has_gb:
        nc.vector.tensor_mul(out=hh0[:], in0=zr0[:, 4:8], in1=brecH[:])
        nc.vector.tensor_add(out=hh0[:], in0=hh0[:], in1=xp4[:, 8:12, :, 0])
        nc.vector.tensor_scalar_max(hh0[:], hh0[:], 0.0)
    else:
        nc.vector.tensor_scalar_max(hh0[:], xp4[:, 8:12, :, 0], 0.0)
    d0 = gt_pool.tile([P, KO, BL], f32, tag="d")
    nc.vector.tensor_mul(out=d0[:], in0=zr0[:, 0:4], in1=hh0[:])
    nc.vector.tensor_sub(out=hs4[:, :, :, 0], in0=hh0[:], in1=d0[:])

    for t in range(1, T):
        r_ps = psum_rec.tile([P, KO * BL], f32, tag="r_ps", name=f"rp_{t}")
        h_ps = psum_rec.tile([P, KO * BL], f32, tag="h_ps", name=f"hp_{t}")
        z_ps = psum_rec.tile([P, KO * BL], f32, tag="z_ps", name=f"zp_{t}")
        nc.tensor.matmul(r_ps[:], lhsT=ident, rhs=xp4[:, 4:8, :, t],
                         start=True, stop=False)
        for m in range(4):
            for ko in range(KO):
                nc.tensor.matmul(r_ps[:, ds(BL * m, BL)],
                                 lhsT=Rb[:, ko, ts(4 + m, P)],
                                 rhs=hs4[:, ko, :, t - 1],
                                 start=False, stop=(ko == KO - 1) and (m == 3))
        for m in range(4):
            for ko in range(KO):
                nc.tensor.matmul(h_ps[:, ds(BL * m, BL)],
                                 lhsT=Rb[:, ko, ts(8 + m, P)],
                                 rhs=hs4[:, ko, :, t - 1],
                                 start=(ko == 0), stop=(ko == KO - 1))
        nc.tensor.matmul(z_ps[:], lhsT=ident, rhs=xp4[:, 0:4, :, t],
                         start=True, stop=False)
        for m in range(4):
            for ko in range(KO):
                nc.tensor.matmul(z_ps[:, ds(BL * m, BL)],
                                 lhsT=Rb[:, ko, ts(m, P)],
                                 rhs=hs4[:, ko, :, t - 1],
                                 start=False, stop=(ko == KO - 1) and (m == 3))
        rr = gt_pool.tile([P, KO, BL], f32, tag="rr", name=f"rr_{t}")
        sigmoid_exp(rr[:], r_ps.rearrange("p (m b) -> p m b", b=BL))
        hh = gt_pool.tile([P, KO, BL], f32, tag="hh", name=f"hh_{t}")
        hp4 = h_ps.rearrange("p (m b) -> p m b", b=BL)
        if has_gb:
            nc.vector.tensor_add(out=hh[:], in0=hp4, in1=brecH[:])
            nc.vector.tensor_mul(out=hh[:], in0=rr[:], in1=hh[:])
        else:
            nc.vector.tensor_mul(out=hh[:], in0=rr[:], in1=hp4)
        nc.vector.tensor_add(out=hh[:], in0=hh[:], in1=xp4[:, 8:12, :, t])
        nc.vector.tensor_scalar_max(hh[:], hh[:], 0.0)
        dd = gt_pool.tile([P, KO, BL], f32, tag="d", name=f"d_{t}")
        nc.vector.tensor_sub(out=dd[:], in0=hs4[:, :, :, t - 1], in1=hh[:])
        zz = gt_pool.tile([P, KO, BL], f32, tag="zz", name=f"zz_{t}")
        sigmoid_exp(zz[:], z_ps.rearrange("p (m b) -> p m b", b=BL))
        nc.vector.tensor_mul(out=dd[:], in0=zz[:], in1=dd[:])
        nc.vector.tensor_add(out=hs4[:, :, :, t], in0=hh[:], in1=dd[:])

        # ---- interleaved auxiliary work for this step ----
        if wload_tasks:
            wload_tasks.pop()()
        pe_busy = False
        # x_proj chunk c: 12 m-groups at steps 32c-14 .. 32c-3
        for c in range(1, XC):
            m = t - (32 * c - 14)
            if 0 <= m < MO3 and debug != "xproj":
                emit_xproj_group(c, m)
                pe_busy = True
        # MLP for quarter q: 8 groups at steps 32q+33 .. 32q+40, AG at +41
        for q in range(NQG - 1):
            g = t - (32 * q + 33)
            if 0 <= g < 2 * KO:
                emit_mlp_group(q, g)
                pe_busy = True
            elif g == 2 * KO:
                emit_gather(q)
        # vocab half-chunk backfill + paced softmax scale/output emission
        if not pe_busy:
            avail = 0
            for q in range(NQG):
                if t >= VOC_START[q]:
                    avail = (q + 1) * NCORES * NJ
            if vstate['chunk'] < avail:
                emit_vocab_half()
        pop_scale_task()
        vstate['tick'] += 1

    if debug == "hseq":
        nc.sync.dma_start(out=dbg_ext, in_=hseqT[:])

    while wload_tasks:
        wload_tasks.pop()()

    # ---- post-scan: last quarter MLP + gather, then drain vocab chunks ----
    for g in range(2 * KO):
        emit_mlp_group(NQG - 1, g)
    emit_gather(NQG - 1)

    if debug == "h2loc":
        nc.sync.dma_start(out=dbg_ext, in_=h2T[:])
    if debug == "h2g":
        for q in range(NQG):
            nc.sync.dma_start(out=dbg_ext[q],
                              in_=h2g[q].rearrange("p ko r bt -> p (ko r bt)"))
    if debug == "w3b":
        nc.sync.dma_start(out=dbg_ext, in_=w3b.rearrange("p k v -> p (k v)"))

    while vstate['chunk'] < NTILE * NJ:
        emit_vocab_half()
        emit_vocab_half()
        pop_scale_task()
        vstate['tick'] += 1
    while vstate['scale_tasks']:
        task, rd, artick = vstate['scale_tasks'].pop(0)
        emit_scale_tile(task)

    voc_stack.close()
    gru_stack.close()
    persist.close()


_BUILD_CACHE = {}


def _get_nc(has_b3: bool, has_gb: bool, debug=None):
    key = (has_b3, has_gb, debug)
    if key not in _BUILD_CACHE:
        _BUILD_CACHE[key] = _build(has_b3, has_gb, debug)
    return _BUILD_CACHE[key]


def _make_in_maps(inputs):
    arrs = {k: np.ascontiguousarray(np.asarray(v, dtype=np.float32))
            for k, v in inputs.items()}
    in_maps = []
    for c in range(NCORES):
        in_maps.append({
            "encoder_input": arrs["encoder_input"][BL * c:BL * (c + 1)],
            "decoder_input": arrs["decoder_input"][BL * c:BL * (c + 1)],
            "gru_kernel": arrs["gru_kernel"],
            "gru_rec_kernel": arrs["gru_rec_kernel"],
            "gru_bias": arrs["gru_bias"],
            "w1": arrs["w1"], "b1": arrs["b1"],
            "w2": arrs["w2"], "b2": arrs["b2"],
            "w3": np.ascontiguousarray(arrs["w3"][:, VS * c:VS * (c + 1)]),
            "b3": np.ascontiguousarray(arrs["b3"][VS * c:VS * (c + 1)]),
        })
    flags = (bool(np.any(arrs["b3"])), bool(np.any(arrs["gru_bias"])))
    return in_maps, flags


def kernel(**inputs):
    global LAST_RESULT
    in_maps, (has_b3, has_gb) = _make_in_maps(inputs)
    nc = _get_nc(has_b3, has_gb)
    res = run_bass_kernel_spmd(nc, in_maps, core_ids=list(range(NCORES)),
                               trace=TRACE, **TRACE_KWARGS)
    LAST_RESULT = res
    full = np.empty((B, T, V), np.float32)
    for c in range(NCORES):
        o = np.asarray(res.results[c]["out"], dtype=np.float32)
        # o[q*8+r, bl*32+tq, v] -> full[4r+bl, 32q+tq, VS*c+v]
        o = o.reshape(NQG, NCORES, BL, TQS, VS)
        full[:, :, VS * c:VS * (c + 1)] = o.transpose(1, 2, 0, 3, 4).reshape(B, T, VS)
    return full
